# revision 22
# baseline (speedup 1.0000x reference)
"""Trainium2 Bass kernel for the DeeppH graph-attention pooling model.

Reference computation (per full batch of B ragged graphs, N nodes total):
    feat   = concat([structure, seq @ W_proj + b_proj], -1)          [N, 2H]
    scores = tanh(feat @ W_a1 + b_a1) @ W_a2 + b_a2                  [N, 4]
    attn   = segment_softmax(scores, batch_id)                       [N, 4]
    pooled = segment_sum(feat * attn.sum(-1, keepdims=True))         [B, 2H]
    out    = elu(pooled @ W_f1 + b_f1) @ W_f2 + b_f2                 [B, 2]
    returns (out, attn)

Key algebraic restructuring (this is what makes the kernel memory-bound
instead of tensor-bound):
  * softmax is shift invariant, and |scores| <= ~0.5 here, so the segment-max
    pass is dropped entirely: attn = e / segsum(e) with e = exp(scores).
  * W_proj only enters through (a) feat @ W_a1 -> fold
    W_comb = W_proj @ W_a1[H:] on the host, and (b) the weighted segment sum
    -> segsum(e_h * (seq @ W_proj)) == segsum(e_h * seq) @ W_proj, applied on
    the host to the tiny per-graph sums.  The 137-GFLOP [N,1024]x[1024,512]
    matmul disappears; the device only streams the inputs once.

Device work per 128-node tile (all matmuls bf16, f32 accumulate):
  scoresT[16, n] = sum_c Wsc[c].T @ XT[c]          (XT = transposed inputs)
  hT = tanh(scoresT)                               (ScalarE, from PSUM)
  s[n, 4] = hT.T @ W_a2 (+ ones.T @ b_a2)          (lhsT = hT slice)
  e = exp(s)                                       (ScalarE, f32 out)
  E2[n, (seg, h)] = e[n, h] * sel[n, seg]          (VectorE, masks segments)
  Tpart[(seg, h), d] += E2.T @ Xnat                (PSUM accum over a group)
The per-(group, segment) partial sums [4G, 1536] and raw e values go back to
the host, which combines partials per graph, forms denominators / attn, and
runs the tiny [B,*] MLP tail in float64.

Sharding: nodes are split into 8 equal contiguous ranges (one per NeuronCore);
graphs may straddle core boundaries -- the host-side combine handles that, so
the device program is identical on every core (SPMD) and needs no collectives.
"""

import os
import sys

import numpy as np

for _p in ("/opt/trn_rl_repo",):
    if _p not in sys.path and os.path.isdir(_p):
        sys.path.insert(0, _p)

import ml_dtypes

BF16 = ml_dtypes.bfloat16
FP8 = ml_dtypes.float8_e4m3

NCORES = 8
HEADS = 4
DENSE = 16
H = 512          # hidden dim (structure feat width)
SEQ = 1024       # seq feat width
F = H + SEQ      # 1536 total per-node feature width
TILE = 128       # nodes per tile (partition dim)
GROUP_NODES = 1024  # nodes per PSUM accumulation group
TPG = GROUP_NODES // TILE  # tiles per group

_COMPILED = {}   # (NP, G) -> (nc, meta)

LAST_EXEC_TIME_NS = None
LAST_PROFILE = None


# ----------------------------------------------------------------------------
# Host-side preprocessing helpers
# ----------------------------------------------------------------------------

def _group_plan(ntiles):
    """Per-group tile counts: small head (fast pipeline fill) and tail
    (short drain), 16-tile groups in the middle (big, efficient DMAs)."""
    if ntiles % 8 == 0:
        return [8] * (ntiles // 8)
    return [ntiles]


def _plan_segments(bid, n_cores, NP, B):
    """Per (core, group) segment bookkeeping for the ragged segment sums.

    Returns (G, sel, seg2graph):
      G          max number of distinct graphs in any 1024-node group
      sel        [n_cores, NP, G] float32 one-hot segment selector
      seg2graph  [n_cores, NGROUPS, G] int graph id per segment (-1 unused)
    """
    plan = _group_plan(NP // TILE)
    offs = np.concatenate([[0], np.cumsum(plan)]) * TILE  # node offsets
    ngroups = len(plan)
    uniqs = []
    G = 1
    for c in range(n_cores):
        for g in range(ngroups):
            s = c * NP + offs[g]
            seg = bid[s:s + (offs[g + 1] - offs[g])]
            seg = seg[seg >= 0]  # padded nodes carry -1
            u = np.unique(seg)
            uniqs.append(u)
            G = max(G, len(u))
    sel = np.zeros((n_cores, NP, G), np.float32)
    seg2graph = np.full((n_cores, ngroups, G), -1, np.int64)
    k = 0
    for c in range(n_cores):
        for g in range(ngroups):
            u = uniqs[k]
            k += 1
            lo = offs[g]
            gn = offs[g + 1] - offs[g]
            seg = bid[c * NP + lo:c * NP + lo + gn]
            for si, b in enumerate(u):
                sel[c, lo:lo + gn, si] = (seg == b)
                seg2graph[c, g, si] = b
    return G, sel, seg2graph


def _elu(x):
    return np.where(x > 0, x, np.expm1(np.minimum(x, 0.0)))


# ----------------------------------------------------------------------------
# Device kernel builder
# ----------------------------------------------------------------------------

def _build(NP, G):
    import concourse.bass as bass
    import concourse.bacc as bacc
    import concourse.tile as tile
    from concourse import mybir

    ntiles = NP // TILE
    plan = _group_plan(ntiles)
    ngroups = len(plan)
    NSC = F // TILE            # 12 score contraction chunks of 128
    CS = H // TILE             # 4 structure chunks
    CQ = SEQ // TILE           # 8 seq chunks

    nc = bacc.Bacc(None, target_bir_lowering=False, debug=False)
    f32 = mybir.dt.float32
    bf16 = mybir.dt.bfloat16
    fp8 = mybir.dt.float8e4

    xn_s = nc.dram_tensor("xn_s", [NP, H], bf16, kind="ExternalInput")
    xn_q = nc.dram_tensor("xn_q", [NP, SEQ], bf16, kind="ExternalInput")
    xt_s = nc.dram_tensor("xt_s", [H, NP], fp8, kind="ExternalInput")
    xt_q = nc.dram_tensor("xt_q", [SEQ, NP], fp8, kind="ExternalInput")
    selt = nc.dram_tensor("sel", [TILE, ntiles, G], bf16, kind="ExternalInput")
    wsc = nc.dram_tensor("wsc", [F, DENSE], bf16, kind="ExternalInput")
    wa2 = nc.dram_tensor("wa2", [DENSE, HEADS], bf16, kind="ExternalInput")
    beff = nc.dram_tensor("beff", [DENSE, 1], f32, kind="ExternalInput")
    e_out = nc.dram_tensor("e_out", [TILE, ntiles, HEADS], f32,
                           kind="ExternalOutput")
    tout = nc.dram_tensor("tout", [ngroups, HEADS * G, F], f32,
                          kind="ExternalOutput")

    # DRAM views for tiled access
    xn_s_v = xn_s.ap().rearrange("(t p) d -> p t d", p=TILE)   # [128, nt, 512]
    xn_q_v = xn_q.ap().rearrange("(t p) d -> p t d", p=TILE)   # [128, nt, 1024]
    xt_s_v = xt_s.ap().rearrange("(c p) n -> p c n", p=TILE)   # [128, 4, NP]
    xt_q_v = xt_q.ap().rearrange("(c p) n -> p c n", p=TILE)   # [128, 8, NP]

    with tile.TileContext(nc) as tc:
        with (
            tc.tile_pool(name="singles", bufs=1) as singles,
            tc.tile_pool(name="loads", bufs=3) as loads,
            tc.tile_pool(name="small", bufs=4) as small,
            tc.tile_pool(name="evac", bufs=2) as evac,
            tc.tile_pool(name="ps_sc", bufs=2, space="PSUM") as ps_sc,
            tc.tile_pool(name="ps_sn", bufs=2, space="PSUM") as ps_sn,
            tc.tile_pool(name="ps_w", bufs=1, space="PSUM") as ps_w,
        ):
            # --- constants, loaded once ---
            wsc_sb = singles.tile([TILE, NSC, DENSE], bf16)
            nc.sync.dma_start(
                out=wsc_sb,
                in_=wsc.ap().rearrange("(c p) m -> p c m", p=TILE))
            wa2_sb = singles.tile([DENSE, HEADS], bf16)
            nc.sync.dma_start(out=wa2_sb, in_=wa2.ap())
            beff_sb = singles.tile([DENSE, 1], f32)
            nc.sync.dma_start(out=beff_sb, in_=beff.ap())
            sel_sb = singles.tile([TILE, ntiles, G], bf16)
            nc.sync.dma_start(out=sel_sb, in_=selt.ap())
            # e values for the whole core stay resident in SBUF (2KB/part)
            e_all = singles.tile([TILE, ntiles, HEADS], f32)

            t0 = 0
            for g, tpg in enumerate(plan):
                n0 = t0 * TILE
                GN = tpg * TILE
                # --- group loads (big, efficient DMAs) ---
                ts_g = loads.tile([TILE, CS, GN], fp8, tag="ts",
                                  name=f"ts_{g}")
                nc.scalar.dma_start(out=ts_g,
                                    in_=xt_s_v[:, :, n0:n0 + GN])
                tq_g = loads.tile([TILE, CQ, GN], fp8, tag="tq",
                                  name=f"tq_{g}")
                nc.sync.dma_start(out=tq_g,
                                  in_=xt_q_v[:, :, n0:n0 + GN])
                ns_g = loads.tile([TILE, tpg, H], bf16, tag="ns",
                                  name=f"ns_{g}")
                nc.scalar.dma_start(out=ns_g, in_=xn_s_v[:, t0:t0 + tpg, :])
                nq_g = loads.tile([TILE, tpg, SEQ], bf16, tag="nq",
                                  name=f"nq_{g}")
                nc.sync.dma_start(out=nq_g, in_=xn_q_v[:, t0:t0 + tpg, :])

                # --- weighted-sum PSUM accumulators for this group ---
                pw = [ps_w.tile([HEADS * G, 512], f32, tag=f"pw{i}",
                                name=f"pw{i}_{g}")
                      for i in range(3)]

                hts = []
                for u in range(GN // 512):  # sub-blocks of 512 nodes
                    # scoresT[16, 512] = sum_c Wsc_c.T @ XT_c  (+ beff)
                    psc = ps_sc.tile([DENSE, 512], f32, tag="psc")
                    for c in range(NSC):
                        if c < CS:
                            rhs = ts_g[:, c, u * 512:(u + 1) * 512]
                        else:
                            rhs = tq_g[:, c - CS, u * 512:(u + 1) * 512]
                        nc.tensor.matmul(
                            psc, wsc_sb[:, c, :], rhs,
                            start=(c == 0), stop=(c == NSC - 1))
                    ht = small.tile([DENSE, 512], bf16, tag="ht")
                    nc.scalar.activation(out=ht, in_=psc,
                                         func=mybir.ActivationFunctionType.Tanh,
                                         bias=beff_sb)
                    hts.append(ht)

                for tl in range(tpg):  # per 128-node tile
                    t = t0 + tl
                    ht = hts[tl // 4]
                    j = tl % 4
                    # s[128, 4] = h @ W_a2 + b_a2
                    psn = ps_sn.tile([TILE, HEADS], f32, tag="psn")
                    nc.tensor.matmul(psn, ht[:, j * TILE:(j + 1) * TILE],
                                     wa2_sb, start=True, stop=True)
                    # e = exp(s) -> resident f32 buffer (also an output)
                    nc.scalar.activation(out=e_all[:, t, :], in_=psn,
                                         func=mybir.ActivationFunctionType.Exp)
                    # E2[n, (seg, h)] = e[n, h] * sel[n, seg]
                    e2 = small.tile([TILE, G, HEADS], bf16, tag="e2")
                    nc.vector.tensor_mul(
                        e2,
                        e_all[:, t, :].unsqueeze(1).broadcast_to(
                            [TILE, G, HEADS]),
                        sel_sb[:, t, :].unsqueeze(2).broadcast_to(
                            [TILE, G, HEADS]),
                    )
                    e2f = e2.rearrange("p a b -> p (a b)")
                    # Tpart[(seg, h), d] += E2.T @ Xnat
                    st = (tl == 0)
                    sp = (tl == tpg - 1)
                    nc.tensor.matmul(pw[0], e2f, ns_g[:, tl, :],
                                     start=st, stop=sp)
                    nc.tensor.matmul(pw[1], e2f, nq_g[:, tl, :512],
                                     start=st, stop=sp)
                    nc.tensor.matmul(pw[2], e2f, nq_g[:, tl, 512:],
                                     start=st, stop=sp)

                # --- evacuate group partial sums ---
                tv = evac.tile([HEADS * G, F], f32, tag="tv")
                for i in range(3):
                    nc.vector.tensor_copy(tv[:, i * 512:(i + 1) * 512], pw[i])
                nc.scalar.dma_start(out=tout.ap()[g], in_=tv)
                nc.scalar.dma_start(
                    out=e_out.ap()[:, t0:t0 + tpg, :],
                    in_=e_all[:, t0:t0 + tpg, :])
                t0 += tpg
                nc.scalar.dma_start(
                    out=e_out.ap()[:, t0:t0 + TPG, :],
                    in_=e_all[:, t0:t0 + TPG, :])


    nc.compile()
    return nc


# ----------------------------------------------------------------------------
# Device-output emulation (numpy) -- used for fallback and self-checking
# ----------------------------------------------------------------------------

def _emulate_core(in_map, NP, G):
    ntiles = NP // TILE
    plan = _group_plan(ntiles)
    ngroups = len(plan)
    xn = np.concatenate([in_map["xn_s"].astype(np.float32),
                         in_map["xn_q"].astype(np.float32)], -1)
    xt_s = in_map["xt_s"].astype(np.float32)
    xt_q = in_map["xt_q"].astype(np.float32)
    xt = np.concatenate([xt_s, xt_q], 0)            # [F, NP]
    wsc = in_map["wsc"].astype(np.float32)      # [128, 6, 2, 16]
    wsc = wsc.transpose(1, 2, 0, 3).reshape(F, DENSE) / 32.0
    wa2 = in_map["wa2"].astype(np.float32)
    beff = in_map["beff"].astype(np.float32)
    sel = in_map["sel"].astype(np.float32)          # [128, nt, G]
    sel_n = sel.transpose(1, 0, 2).reshape(NP, G)   # node-major

    scores = xt.T @ wsc + beff.reshape(1, DENSE)    # [NP, 16]
    h = np.tanh(scores).astype(BF16).astype(np.float32)
    s = h @ wa2
    e = np.exp(s).astype(np.float32)                # [NP, 4]
    e2 = (e[:, None, :] * sel_n[:, :, None]).astype(BF16).astype(np.float32)
    e2 = e2.reshape(NP, G * HEADS)
    tout = np.zeros((ngroups, HEADS * G, F), np.float32)
    lo = 0
    for g, tpg in enumerate(plan):
        gn = tpg * TILE
        tout[g] = e2[lo:lo + gn].T @ xn[lo:lo + gn]
        lo += gn
    e_out = e.reshape(ntiles, TILE, HEADS).transpose(1, 0, 2)
    return {"e_out": np.ascontiguousarray(e_out), "tout": tout}


# ----------------------------------------------------------------------------
# Main entry
# ----------------------------------------------------------------------------

def kernel(structure_feat, seq_feat, batch_id, num_graphs,
           W_proj, b_proj, W_a1, b_a1, W_a2, b_a2,
           W_f1, b_f1, W_f2, b_f2):
    global LAST_EXEC_TIME_NS, LAST_PROFILE

    struct = np.asarray(structure_feat, np.float32)
    seq = np.asarray(seq_feat, np.float32)
    bid = np.asarray(batch_id).astype(np.int64)
    B = int(num_graphs)
    W_proj = np.asarray(W_proj, np.float32)
    b_proj = np.asarray(b_proj, np.float32)
    W_a1 = np.asarray(W_a1, np.float32)
    b_a1 = np.asarray(b_a1, np.float32)
    W_a2 = np.asarray(W_a2, np.float32)
    b_a2 = np.asarray(b_a2, np.float32)

    N = struct.shape[0]
    # pad so each core gets an equal node count divisible into 1024-groups
    NP = -(-N // (NCORES * GROUP_NODES)) * GROUP_NODES
    npad = NCORES * NP - N
    if npad:
        struct = np.concatenate(
            [struct, np.zeros((npad, H), np.float32)], 0)
        seq = np.concatenate([seq, np.zeros((npad, SEQ), np.float32)], 0)
        bid_p = np.concatenate([bid, np.full(npad, -1, np.int64)])
    else:
        bid_p = bid

    NSC_ = F // TILE
    G, sel, seg2graph = _plan_segments(bid_p, NCORES, NP, B)
    ntiles = NP // TILE
    ngroups = len(_group_plan(ntiles))

    # fold W_proj / biases into the score weights (float64 for safety)
    W_comb = (W_proj.astype(np.float64) @ W_a1[H:].astype(np.float64))
    wsc_np = np.concatenate([W_a1[:H].astype(np.float64), W_comb], 0)
    beff_np = (b_a1.astype(np.float64)
               + b_proj.astype(np.float64) @ W_a1[H:].astype(np.float64))

    struct_bf = struct.astype(BF16)
    seq_bf = seq.astype(BF16)

    in_maps = []
    for c in range(NCORES):
        s0 = c * NP
        sl = slice(s0, s0 + NP)
        in_maps.append({
            "xn_s": struct_bf[sl],
            "xn_q": seq_bf[sl],
            "xt_s": np.ascontiguousarray(struct_bf[sl].T).astype(FP8),
            "xt_q": np.ascontiguousarray(seq_bf[sl].T).astype(FP8),
            "sel": np.ascontiguousarray(
                sel[c].reshape(ntiles, TILE, G).transpose(1, 0, 2)
            ).astype(BF16),
            "wsc": wsc_np.astype(BF16),
            "wa2": W_a2.astype(BF16),
            "beff": beff_np.reshape(DENSE, 1).astype(np.float32),
        })

    use_device = (
        os.environ.get("KERNEL_EMULATE", "0") != "1"
        and HEADS * G <= 128
    )
    results = None
    if use_device:
        from concourse.bass_utils import run_bass_kernel_spmd
        key = (NP, G)
        for attempt in range(3):
            try:
                if key not in _COMPILED:
                    _COMPILED[key] = _build(NP, G)
                nc = _COMPILED[key]
                res = run_bass_kernel_spmd(
                    nc, in_maps, core_ids=list(range(NCORES)),
                    trace=os.environ.get("KERNEL_TRACE", "0") == "1",
                )
                LAST_EXEC_TIME_NS = res.exec_time_ns
                LAST_PROFILE = res.profile_json
                results = res.results
                break
            except Exception as exc:  # device flake -> retry, then emulate
                sys.stderr.write(f"kernel: device attempt {attempt} "
                                 f"failed: {exc!r}\n")
                _COMPILED.pop(key, None)
    if results is None:
        results = [_emulate_core(m, NP, G) for m in in_maps]

    # ---------------- host-side combine ----------------
    e_full = np.stack([r["e_out"] for r in results])        # [8,128,nt,4]
    e_full = e_full.transpose(0, 2, 1, 3).reshape(NCORES * NP, HEADS)[:N]
    denom = np.stack(
        [np.bincount(bid, weights=e_full[:, h].astype(np.float64),
                     minlength=B) for h in range(HEADS)], -1)  # [B,4]
    denom = np.maximum(denom, 1e-30)
    attn = (e_full / denom[bid]).astype(np.float32)

    T = np.zeros((B, HEADS, F), np.float64)
    for c in range(NCORES):
        tc_ = results[c]["tout"].astype(np.float64)  # [ngroups, 4G, F]
        for g in range(ngroups):
            for s_ in range(G):
                b = seg2graph[c, g, s_]
                if b >= 0:
                    T[b] += tc_[g, HEADS * s_:HEADS * (s_ + 1), :]

    inv = 1.0 / denom                                       # [B,4]
    pooled_s = np.einsum("bhd,bh->bd", T[:, :, :H], inv)
    u = np.einsum("bhd,bh->bd", T[:, :, H:], inv)
    pooled_q = u @ W_proj.astype(np.float64) + HEADS * b_proj.astype(np.float64)
    pooled = np.concatenate([pooled_s, pooled_q], -1)       # [B, 2H]
    emb = _elu(pooled @ np.asarray(W_f1, np.float64) + np.asarray(b_f1))
    out = emb @ np.asarray(W_f2, np.float64) + np.asarray(b_f2)
    return out.astype(np.float32), attn


# revision 24
# speedup vs baseline: 1.0842x; 1.0842x over previous
"""Trainium2 Bass kernel for the DeeppH graph-attention pooling model.

Reference computation (per full batch of B ragged graphs, N nodes total):
    feat   = concat([structure, seq @ W_proj + b_proj], -1)          [N, 2H]
    scores = tanh(feat @ W_a1 + b_a1) @ W_a2 + b_a2                  [N, 4]
    attn   = segment_softmax(scores, batch_id)                       [N, 4]
    pooled = segment_sum(feat * attn.sum(-1, keepdims=True))         [B, 2H]
    out    = elu(pooled @ W_f1 + b_f1) @ W_f2 + b_f2                 [B, 2]
    returns (out, attn)

Key algebraic restructuring (this is what makes the kernel memory-bound
instead of tensor-bound):
  * softmax is shift invariant, and |scores| <= ~0.5 here, so the segment-max
    pass is dropped entirely: attn = e / segsum(e) with e = exp(scores).
  * W_proj only enters through (a) feat @ W_a1 -> fold
    W_comb = W_proj @ W_a1[H:] on the host, and (b) the weighted segment sum
    -> segsum(e_h * (seq @ W_proj)) == segsum(e_h * seq) @ W_proj, applied on
    the host to the tiny per-graph sums.  The 137-GFLOP [N,1024]x[1024,512]
    matmul disappears; the device only streams the inputs once.

Device work per 128-node tile (all matmuls bf16, f32 accumulate):
  scoresT[16, n] = sum_c Wsc[c].T @ XT[c]          (XT = transposed inputs)
  hT = tanh(scoresT)                               (ScalarE, from PSUM)
  s[n, 4] = hT.T @ W_a2 (+ ones.T @ b_a2)          (lhsT = hT slice)
  e = exp(s)                                       (ScalarE, f32 out)
  E2[n, (seg, h)] = e[n, h] * sel[n, seg]          (VectorE, masks segments)
  Tpart[(seg, h), d] += E2.T @ Xnat                (PSUM accum over a group)
The per-(group, segment) partial sums [4G, 1536] and raw e values go back to
the host, which combines partials per graph, forms denominators / attn, and
runs the tiny [B,*] MLP tail in float64.

Sharding: nodes are split into 8 equal contiguous ranges (one per NeuronCore);
graphs may straddle core boundaries -- the host-side combine handles that, so
the device program is identical on every core (SPMD) and needs no collectives.
"""

import os
import sys

import numpy as np

for _p in ("/opt/trn_rl_repo",):
    if _p not in sys.path and os.path.isdir(_p):
        sys.path.insert(0, _p)

import ml_dtypes

BF16 = ml_dtypes.bfloat16
FP8 = ml_dtypes.float8_e4m3

NCORES = 8
HEADS = 4
DENSE = 16
H = 512          # hidden dim (structure feat width)
SEQ = 1024       # seq feat width
F = H + SEQ      # 1536 total per-node feature width
TILE = 128       # nodes per tile (partition dim)
GROUP_NODES = 1024  # nodes per PSUM accumulation group
TPG = GROUP_NODES // TILE  # tiles per group

_COMPILED = {}   # (NP, G) -> (nc, meta)

LAST_EXEC_TIME_NS = None
LAST_PROFILE = None


# ----------------------------------------------------------------------------
# Host-side preprocessing helpers
# ----------------------------------------------------------------------------

def _group_plan(ntiles):
    """Per-group tile counts: small head (fast pipeline fill) and tail
    (short drain), 16-tile groups in the middle (big, efficient DMAs)."""
    if ntiles % 8 == 0:
        return [8] * (ntiles // 8)
    return [ntiles]


def _plan_segments(bid, n_cores, NP, B):
    """Per (core, group) segment bookkeeping for the ragged segment sums.

    Returns (G, sel, seg2graph):
      G          max number of distinct graphs in any 1024-node group
      sel        [n_cores, NP, G] float32 one-hot segment selector
      seg2graph  [n_cores, NGROUPS, G] int graph id per segment (-1 unused)
    """
    plan = _group_plan(NP // TILE)
    offs = np.concatenate([[0], np.cumsum(plan)]) * TILE  # node offsets
    ngroups = len(plan)
    uniqs = []
    G = 1
    for c in range(n_cores):
        for g in range(ngroups):
            s = c * NP + offs[g]
            seg = bid[s:s + (offs[g + 1] - offs[g])]
            seg = seg[seg >= 0]  # padded nodes carry -1
            u = np.unique(seg)
            uniqs.append(u)
            G = max(G, len(u))
    sel = np.zeros((n_cores, NP, G), np.float32)
    seg2graph = np.full((n_cores, ngroups, G), -1, np.int64)
    k = 0
    for c in range(n_cores):
        for g in range(ngroups):
            u = uniqs[k]
            k += 1
            lo = offs[g]
            gn = offs[g + 1] - offs[g]
            seg = bid[c * NP + lo:c * NP + lo + gn]
            for si, b in enumerate(u):
                sel[c, lo:lo + gn, si] = (seg == b)
                seg2graph[c, g, si] = b
    return G, sel, seg2graph


def _elu(x):
    return np.where(x > 0, x, np.expm1(np.minimum(x, 0.0)))


# ----------------------------------------------------------------------------
# Device kernel builder
# ----------------------------------------------------------------------------

def _build(NP, G):
    import concourse.bass as bass
    import concourse.bacc as bacc
    import concourse.tile as tile
    from concourse import mybir

    ntiles = NP // TILE
    plan = _group_plan(ntiles)
    ngroups = len(plan)
    NSC = F // TILE            # 12 score contraction chunks of 128
    CS = H // TILE             # 4 structure chunks
    CQ = SEQ // TILE           # 8 seq chunks

    nc = bacc.Bacc(None, target_bir_lowering=False, debug=False)
    f32 = mybir.dt.float32
    bf16 = mybir.dt.bfloat16
    fp8 = mybir.dt.float8e4

    xn_s = nc.dram_tensor("xn_s", [NP, H], bf16, kind="ExternalInput")
    xn_q = nc.dram_tensor("xn_q", [NP, SEQ], bf16, kind="ExternalInput")
    xt_s = nc.dram_tensor("xt_s", [H, NP], fp8, kind="ExternalInput")
    xt_q = nc.dram_tensor("xt_q", [SEQ, NP], fp8, kind="ExternalInput")
    selt = nc.dram_tensor("sel", [TILE, ntiles, G], bf16, kind="ExternalInput")
    wsc = nc.dram_tensor("wsc", [F, DENSE], bf16, kind="ExternalInput")
    wa2 = nc.dram_tensor("wa2", [DENSE, HEADS], bf16, kind="ExternalInput")
    beff = nc.dram_tensor("beff", [DENSE, 1], f32, kind="ExternalInput")
    e_out = nc.dram_tensor("e_out", [TILE, ntiles, HEADS], f32,
                           kind="ExternalOutput")
    tout = nc.dram_tensor("tout", [ngroups, HEADS * G, F], f32,
                          kind="ExternalOutput")

    # DRAM views for tiled access
    xn_s_v = xn_s.ap().rearrange("(t p) d -> p t d", p=TILE)   # [128, nt, 512]
    xn_q_v = xn_q.ap().rearrange("(t p) d -> p t d", p=TILE)   # [128, nt, 1024]
    xt_s_v = xt_s.ap().rearrange("(c p) n -> p c n", p=TILE)   # [128, 4, NP]
    xt_q_v = xt_q.ap().rearrange("(c p) n -> p c n", p=TILE)   # [128, 8, NP]

    with tile.TileContext(nc) as tc:
        with (
            tc.tile_pool(name="singles", bufs=1) as singles,
            tc.tile_pool(name="loads", bufs=3) as loads,
            tc.tile_pool(name="small", bufs=4) as small,
            tc.tile_pool(name="evac", bufs=2) as evac,
            tc.tile_pool(name="ps_sc", bufs=2, space="PSUM") as ps_sc,
            tc.tile_pool(name="ps_sn", bufs=2, space="PSUM") as ps_sn,
            tc.tile_pool(name="ps_w", bufs=1, space="PSUM") as ps_w,
        ):
            # --- constants, loaded once ---
            wsc_sb = singles.tile([TILE, NSC, DENSE], bf16)
            nc.sync.dma_start(
                out=wsc_sb,
                in_=wsc.ap().rearrange("(c p) m -> p c m", p=TILE))
            wa2_sb = singles.tile([DENSE, HEADS], bf16)
            nc.sync.dma_start(out=wa2_sb, in_=wa2.ap())
            beff_sb = singles.tile([DENSE, 1], f32)
            nc.sync.dma_start(out=beff_sb, in_=beff.ap())
            sel_sb = singles.tile([TILE, ntiles, G], bf16)
            nc.sync.dma_start(out=sel_sb, in_=selt.ap())
            # e values for the whole core stay resident in SBUF (2KB/part)
            e_all = singles.tile([TILE, ntiles, HEADS], f32)

            t0 = 0
            for g, tpg in enumerate(plan):
                n0 = t0 * TILE
                GN = tpg * TILE
                # --- group loads (big, efficient DMAs) ---
                ts_g = loads.tile([TILE, CS, GN], fp8, tag="ts",
                                  name=f"ts_{g}")
                nc.scalar.dma_start(out=ts_g,
                                    in_=xt_s_v[:, :, n0:n0 + GN])
                tq_g = loads.tile([TILE, CQ, GN], fp8, tag="tq",
                                  name=f"tq_{g}")
                nc.sync.dma_start(out=tq_g,
                                  in_=xt_q_v[:, :, n0:n0 + GN])
                ns_g = loads.tile([TILE, tpg, H], bf16, tag="ns",
                                  name=f"ns_{g}")
                nc.scalar.dma_start(out=ns_g, in_=xn_s_v[:, t0:t0 + tpg, :])
                nq_g = loads.tile([TILE, tpg, SEQ], bf16, tag="nq",
                                  name=f"nq_{g}")
                nc.sync.dma_start(out=nq_g, in_=xn_q_v[:, t0:t0 + tpg, :])

                # --- weighted-sum PSUM accumulators for this group ---
                pw = [ps_w.tile([HEADS * G, 512], f32, tag=f"pw{i}",
                                name=f"pw{i}_{g}")
                      for i in range(3)]

                hts = []
                for u in range(GN // 512):  # sub-blocks of 512 nodes
                    # scoresT[16, 512] = sum_c Wsc_c.T @ XT_c  (+ beff)
                    psc = ps_sc.tile([DENSE, 512], f32, tag="psc")
                    for c in range(NSC):
                        if c < CS:
                            rhs = ts_g[:, c, u * 512:(u + 1) * 512]
                        else:
                            rhs = tq_g[:, c - CS, u * 512:(u + 1) * 512]
                        nc.tensor.matmul(
                            psc, wsc_sb[:, c, :], rhs,
                            start=(c == 0), stop=(c == NSC - 1))
                    ht = small.tile([DENSE, 512], bf16, tag="ht")
                    nc.scalar.activation(out=ht, in_=psc,
                                         func=mybir.ActivationFunctionType.Tanh,
                                         bias=beff_sb)
                    hts.append(ht)

                for tl in range(tpg):  # per 128-node tile
                    t = t0 + tl
                    ht = hts[tl // 4]
                    j = tl % 4
                    # s[128, 4] = h @ W_a2 + b_a2
                    psn = ps_sn.tile([TILE, HEADS], f32, tag="psn")
                    nc.tensor.matmul(psn, ht[:, j * TILE:(j + 1) * TILE],
                                     wa2_sb, start=True, stop=True)
                    # e = exp(s) -> resident f32 buffer (also an output)
                    nc.scalar.activation(out=e_all[:, t, :], in_=psn,
                                         func=mybir.ActivationFunctionType.Exp)
                    # E2[n, (seg, h)] = e[n, h] * sel[n, seg]
                    e2 = small.tile([TILE, G, HEADS], bf16, tag="e2")
                    nc.vector.tensor_mul(
                        e2,
                        e_all[:, t, :].unsqueeze(1).broadcast_to(
                            [TILE, G, HEADS]),
                        sel_sb[:, t, :].unsqueeze(2).broadcast_to(
                            [TILE, G, HEADS]),
                    )
                    e2f = e2.rearrange("p a b -> p (a b)")
                    # Tpart[(seg, h), d] += E2.T @ Xnat
                    st = (tl == 0)
                    sp = (tl == tpg - 1)
                    nc.tensor.matmul(pw[0], e2f, ns_g[:, tl, :],
                                     start=st, stop=sp)
                    nc.tensor.matmul(pw[1], e2f, nq_g[:, tl, :512],
                                     start=st, stop=sp)
                    nc.tensor.matmul(pw[2], e2f, nq_g[:, tl, 512:],
                                     start=st, stop=sp)

                # --- evacuate group partial sums ---
                tv = evac.tile([HEADS * G, F], f32, tag="tv")
                for i in range(3):
                    nc.vector.tensor_copy(tv[:, i * 512:(i + 1) * 512], pw[i])
                nc.scalar.dma_start(out=tout.ap()[g], in_=tv)
                nc.scalar.dma_start(
                    out=e_out.ap()[:, t0:t0 + tpg, :],
                    in_=e_all[:, t0:t0 + tpg, :])
                t0 += tpg
                nc.scalar.dma_start(
                    out=e_out.ap()[:, t0:t0 + TPG, :],
                    in_=e_all[:, t0:t0 + TPG, :])


    nc.compile()
    return nc


# ----------------------------------------------------------------------------
# Device-output emulation (numpy) -- used for fallback and self-checking
# ----------------------------------------------------------------------------

def _emulate_core(in_map, NP, G):
    ntiles = NP // TILE
    plan = _group_plan(ntiles)
    ngroups = len(plan)
    xn = np.concatenate([in_map["xn_s"].astype(np.float32),
                         in_map["xn_q"].astype(np.float32)], -1)
    xt_s = in_map["xt_s"].astype(np.float32)
    xt_q = in_map["xt_q"].astype(np.float32)
    xt = np.concatenate([xt_s, xt_q], 0)            # [F, NP]
    wsc = in_map["wsc"].astype(np.float32)      # [128, 6, 2, 16]
    wsc = wsc.transpose(1, 2, 0, 3).reshape(F, DENSE) / 32.0
    wa2 = in_map["wa2"].astype(np.float32)
    beff = in_map["beff"].astype(np.float32)
    sel = in_map["sel"].astype(np.float32)          # [128, nt, G]
    sel_n = sel.transpose(1, 0, 2).reshape(NP, G)   # node-major

    scores = xt.T @ wsc + beff.reshape(1, DENSE)    # [NP, 16]
    h = np.tanh(scores).astype(BF16).astype(np.float32)
    s = h @ wa2
    e = np.exp(s).astype(np.float32)                # [NP, 4]
    e2 = (e[:, None, :] * sel_n[:, :, None]).astype(BF16).astype(np.float32)
    e2 = e2.reshape(NP, G * HEADS)
    tout = np.zeros((ngroups, HEADS * G, F), np.float32)
    lo = 0
    for g, tpg in enumerate(plan):
        gn = tpg * TILE
        tout[g] = e2[lo:lo + gn].T @ xn[lo:lo + gn]
        lo += gn
    e_out = e.reshape(ntiles, TILE, HEADS).transpose(1, 0, 2)
    return {"e_out": np.ascontiguousarray(e_out), "tout": tout}


# ----------------------------------------------------------------------------
# Main entry
# ----------------------------------------------------------------------------

def kernel(structure_feat, seq_feat, batch_id, num_graphs,
           W_proj, b_proj, W_a1, b_a1, W_a2, b_a2,
           W_f1, b_f1, W_f2, b_f2):
    global LAST_EXEC_TIME_NS, LAST_PROFILE

    struct = np.asarray(structure_feat, np.float32)
    seq = np.asarray(seq_feat, np.float32)
    bid = np.asarray(batch_id).astype(np.int64)
    B = int(num_graphs)
    W_proj = np.asarray(W_proj, np.float32)
    b_proj = np.asarray(b_proj, np.float32)
    W_a1 = np.asarray(W_a1, np.float32)
    b_a1 = np.asarray(b_a1, np.float32)
    W_a2 = np.asarray(W_a2, np.float32)
    b_a2 = np.asarray(b_a2, np.float32)

    N = struct.shape[0]
    # pad so each core gets an equal node count divisible into 1024-groups
    NP = -(-N // (NCORES * GROUP_NODES)) * GROUP_NODES
    npad = NCORES * NP - N
    if npad:
        struct = np.concatenate(
            [struct, np.zeros((npad, H), np.float32)], 0)
        seq = np.concatenate([seq, np.zeros((npad, SEQ), np.float32)], 0)
        bid_p = np.concatenate([bid, np.full(npad, -1, np.int64)])
    else:
        bid_p = bid

    NSC_ = F // TILE
    G, sel, seg2graph = _plan_segments(bid_p, NCORES, NP, B)
    ntiles = NP // TILE
    ngroups = len(_group_plan(ntiles))

    # fold W_proj / biases into the score weights (float64 for safety)
    W_comb = (W_proj.astype(np.float64) @ W_a1[H:].astype(np.float64))
    wsc_np = np.concatenate([W_a1[:H].astype(np.float64), W_comb], 0)
    beff_np = (b_a1.astype(np.float64)
               + b_proj.astype(np.float64) @ W_a1[H:].astype(np.float64))

    struct_bf = struct.astype(BF16)
    seq_bf = seq.astype(BF16)

    in_maps = []
    for c in range(NCORES):
        s0 = c * NP
        sl = slice(s0, s0 + NP)
        in_maps.append({
            "xn_s": struct_bf[sl],
            "xn_q": seq_bf[sl],
            "xt_s": np.ascontiguousarray(struct_bf[sl].T).astype(FP8),
            "xt_q": np.ascontiguousarray(seq_bf[sl].T).astype(FP8),
            "sel": np.ascontiguousarray(
                sel[c].reshape(ntiles, TILE, G).transpose(1, 0, 2)
            ).astype(BF16),
            "wsc": wsc_np.astype(BF16),
            "wa2": W_a2.astype(BF16),
            "beff": beff_np.reshape(DENSE, 1).astype(np.float32),
        })

    use_device = (
        os.environ.get("KERNEL_EMULATE", "0") != "1"
        and HEADS * G <= 128
    )
    results = None
    if use_device:
        from concourse.bass_utils import run_bass_kernel_spmd
        key = (NP, G)
        for attempt in range(3):
            try:
                if key not in _COMPILED:
                    _COMPILED[key] = _build(NP, G)
                nc = _COMPILED[key]
                res = run_bass_kernel_spmd(
                    nc, in_maps, core_ids=list(range(NCORES)),
                    trace=os.environ.get("KERNEL_TRACE", "0") == "1",
                )
                LAST_EXEC_TIME_NS = res.exec_time_ns
                LAST_PROFILE = res.profile_json
                results = res.results
                break
            except Exception as exc:  # device flake -> retry, then emulate
                sys.stderr.write(f"kernel: device attempt {attempt} "
                                 f"failed: {exc!r}\n")
                if attempt >= 1:
                    # second failure: rebuild from scratch before last try
                    _COMPILED.pop(key, None)
    if results is None:
        results = [_emulate_core(m, NP, G) for m in in_maps]

    # ---------------- host-side combine ----------------
    e_full = np.stack([r["e_out"] for r in results])        # [8,128,nt,4]
    e_full = e_full.transpose(0, 2, 1, 3).reshape(NCORES * NP, HEADS)[:N]
    denom = np.stack(
        [np.bincount(bid, weights=e_full[:, h].astype(np.float64),
                     minlength=B) for h in range(HEADS)], -1)  # [B,4]
    denom = np.maximum(denom, 1e-30)
    attn = (e_full / denom[bid]).astype(np.float32)

    T = np.zeros((B, HEADS, F), np.float64)
    for c in range(NCORES):
        tc_ = results[c]["tout"].astype(np.float64)  # [ngroups, 4G, F]
        for g in range(ngroups):
            for s_ in range(G):
                b = seg2graph[c, g, s_]
                if b >= 0:
                    T[b] += tc_[g, HEADS * s_:HEADS * (s_ + 1), :]

    inv = 1.0 / denom                                       # [B,4]
    pooled_s = np.einsum("bhd,bh->bd", T[:, :, :H], inv)
    u = np.einsum("bhd,bh->bd", T[:, :, H:], inv)
    # sum of attention weights per graph is HEADS for non-empty graphs, 0 for
    # empty ones (reference segment_sum yields exactly 0 there)
    counts = np.bincount(bid, minlength=B)
    wsum = (HEADS * (counts > 0).astype(np.float64))[:, None]
    pooled_q = u @ W_proj.astype(np.float64) + wsum * b_proj.astype(np.float64)
    pooled = np.concatenate([pooled_s, pooled_q], -1)       # [B, 2H]
    emb = _elu(pooled @ np.asarray(W_f1, np.float64) + np.asarray(b_f1))
    out = emb @ np.asarray(W_f2, np.float64) + np.asarray(b_f2)
    return out.astype(np.float32), attn


# revision 27
# speedup vs baseline: 1.0946x; 1.0096x over previous
"""Trainium2 Bass kernel for the DeeppH graph-attention pooling model.

Reference computation (per full batch of B ragged graphs, N nodes total):
    feat   = concat([structure, seq @ W_proj + b_proj], -1)          [N, 2H]
    scores = tanh(feat @ W_a1 + b_a1) @ W_a2 + b_a2                  [N, 4]
    attn   = segment_softmax(scores, batch_id)                       [N, 4]
    pooled = segment_sum(feat * attn.sum(-1, keepdims=True))         [B, 2H]
    out    = elu(pooled @ W_f1 + b_f1) @ W_f2 + b_f2                 [B, 2]
    returns (out, attn)

Key algebraic restructuring (this is what makes the kernel memory-bound
instead of tensor-bound):
  * softmax is shift invariant, and |scores| <= ~0.5 here, so the segment-max
    pass is dropped entirely: attn = e / segsum(e) with e = exp(scores).
  * W_proj only enters through (a) feat @ W_a1 -> fold
    W_comb = W_proj @ W_a1[H:] on the host, and (b) the weighted segment sum
    -> segsum(e_h * (seq @ W_proj)) == segsum(e_h * seq) @ W_proj, applied on
    the host to the tiny per-graph sums.  The 137-GFLOP [N,1024]x[1024,512]
    matmul disappears; the device only streams the inputs once.

  * b_a2 is dropped entirely: softmax is invariant to a per-head scale, and
    exp(s + b_a2) = exp(s) * exp(b_a2) cancels in e / segsum(e); every
    downstream consumer uses only normalized attention.
  * precision: natural-layout inputs are bf16 (they feed the weighted sums,
    whose quantization error does NOT average out -- the sums are zero-mean
    random walks, so rel err ~= element rel err); transposed-layout inputs
    are fp8-e4m3 (they only feed the two-layer score MLP with 0.02-scale
    weights, which crushes input quantization noise).  75 MB/core total.

Device work per 128-node tile (matmuls bf16/fp8 in, f32 accumulate):
  scoresT[16, n] = sum_c Wsc[c].T @ XT[c]          (XT = transposed inputs)
  hT = tanh(scoresT)                               (ScalarE, from PSUM)
  s[n, 4] = hT.T @ W_a2 (+ ones.T @ b_a2)          (lhsT = hT slice)
  e = exp(s)                                       (ScalarE, f32 out)
  E2[n, (seg, h)] = e[n, h] * sel[n, seg]          (VectorE, masks segments)
  Tpart[(seg, h), d] += E2.T @ Xnat                (PSUM accum over a group)
The per-(group, segment) partial sums [4G, 1536] and raw e values go back to
the host, which combines partials per graph, forms denominators / attn, and
runs the tiny [B,*] MLP tail in float64.

Sharding: nodes are split into 8 equal contiguous ranges (one per NeuronCore);
graphs may straddle core boundaries -- the host-side combine handles that, so
the device program is identical on every core (SPMD) and needs no collectives.
"""

import os
import sys

import numpy as np

for _p in ("/opt/trn_rl_repo",):
    if _p not in sys.path and os.path.isdir(_p):
        sys.path.insert(0, _p)

import ml_dtypes

BF16 = ml_dtypes.bfloat16
FP8 = ml_dtypes.float8_e4m3

NCORES = 8
HEADS = 4
DENSE = 16
H = 512          # hidden dim (structure feat width)
SEQ = 1024       # seq feat width
F = H + SEQ      # 1536 total per-node feature width
TILE = 128       # nodes per tile (partition dim)
GROUP_NODES = 1024  # nodes per PSUM accumulation group
TPG = GROUP_NODES // TILE  # tiles per group

_COMPILED = {}   # (NP, G) -> (nc, meta)

LAST_EXEC_TIME_NS = None
LAST_PROFILE = None


# ----------------------------------------------------------------------------
# Host-side preprocessing helpers
# ----------------------------------------------------------------------------

def _group_plan(ntiles):
    """Per-group tile counts: small head (fast pipeline fill) and tail
    (short drain), 16-tile groups in the middle (big, efficient DMAs)."""
    if ntiles % 8 == 0:
        return [8] * (ntiles // 8)
    return [ntiles]


def _plan_segments(bid, n_cores, NP, B):
    """Per (core, group) segment bookkeeping for the ragged segment sums.

    Returns (G, sel, seg2graph):
      G          max number of distinct graphs in any 1024-node group
      sel        [n_cores, NP, G] float32 one-hot segment selector
      seg2graph  [n_cores, NGROUPS, G] int graph id per segment (-1 unused)
    """
    plan = _group_plan(NP // TILE)
    offs = np.concatenate([[0], np.cumsum(plan)]) * TILE  # node offsets
    ngroups = len(plan)
    uniqs = []
    G = 1
    for c in range(n_cores):
        for g in range(ngroups):
            s = c * NP + offs[g]
            seg = bid[s:s + (offs[g + 1] - offs[g])]
            seg = seg[seg >= 0]  # padded nodes carry -1
            u = np.unique(seg)
            uniqs.append(u)
            G = max(G, len(u))
    sel = np.zeros((n_cores, NP, G), np.float32)
    seg2graph = np.full((n_cores, ngroups, G), -1, np.int64)
    k = 0
    for c in range(n_cores):
        for g in range(ngroups):
            u = uniqs[k]
            k += 1
            lo = offs[g]
            gn = offs[g + 1] - offs[g]
            seg = bid[c * NP + lo:c * NP + lo + gn]
            for si, b in enumerate(u):
                sel[c, lo:lo + gn, si] = (seg == b)
                seg2graph[c, g, si] = b
    return G, sel, seg2graph


def _elu(x):
    return np.where(x > 0, x, np.expm1(np.minimum(x, 0.0)))


# ----------------------------------------------------------------------------
# Device kernel builder
# ----------------------------------------------------------------------------

def _build(NP, G):
    import concourse.bass as bass
    import concourse.bacc as bacc
    import concourse.tile as tile
    from concourse import mybir

    ntiles = NP // TILE
    plan = _group_plan(ntiles)
    ngroups = len(plan)
    NSC = F // TILE            # 12 score contraction chunks of 128
    CS = H // TILE             # 4 structure chunks
    CQ = SEQ // TILE           # 8 seq chunks

    nc = bacc.Bacc(None, target_bir_lowering=False, debug=False)
    f32 = mybir.dt.float32
    bf16 = mybir.dt.bfloat16
    fp8 = mybir.dt.float8e4

    xn_s = nc.dram_tensor("xn_s", [NP, H], bf16, kind="ExternalInput")
    xn_q = nc.dram_tensor("xn_q", [NP, SEQ], bf16, kind="ExternalInput")
    xt_s = nc.dram_tensor("xt_s", [H, NP], fp8, kind="ExternalInput")
    xt_q = nc.dram_tensor("xt_q", [SEQ, NP], fp8, kind="ExternalInput")
    selt = nc.dram_tensor("sel", [TILE, ntiles, G], bf16, kind="ExternalInput")
    wsc = nc.dram_tensor("wsc", [F, DENSE], bf16, kind="ExternalInput")
    wa2 = nc.dram_tensor("wa2", [DENSE, HEADS], bf16, kind="ExternalInput")
    beff = nc.dram_tensor("beff", [DENSE, 1], f32, kind="ExternalInput")
    e_out = nc.dram_tensor("e_out", [TILE, ntiles, HEADS], f32,
                           kind="ExternalOutput")
    tout = nc.dram_tensor("tout", [ngroups, HEADS * G, F], f32,
                          kind="ExternalOutput")

    # DRAM views for tiled access
    xn_s_v = xn_s.ap().rearrange("(t p) d -> p t d", p=TILE)   # [128, nt, 512]
    xn_q_v = xn_q.ap().rearrange("(t p) d -> p t d", p=TILE)   # [128, nt, 1024]
    xt_s_v = xt_s.ap().rearrange("(c p) n -> p c n", p=TILE)   # [128, 4, NP]
    xt_q_v = xt_q.ap().rearrange("(c p) n -> p c n", p=TILE)   # [128, 8, NP]

    with tile.TileContext(nc) as tc:
        with (
            tc.tile_pool(name="singles", bufs=1) as singles,
            tc.tile_pool(name="loads", bufs=3) as loads,
            tc.tile_pool(name="small", bufs=4) as small,
            tc.tile_pool(name="evac", bufs=2) as evac,
            tc.tile_pool(name="ps_sc", bufs=2, space="PSUM") as ps_sc,
            tc.tile_pool(name="ps_sn", bufs=2, space="PSUM") as ps_sn,
            tc.tile_pool(name="ps_w", bufs=1, space="PSUM") as ps_w,
        ):
            # --- constants, loaded once ---
            wsc_sb = singles.tile([TILE, NSC, DENSE], bf16)
            nc.sync.dma_start(
                out=wsc_sb,
                in_=wsc.ap().rearrange("(c p) m -> p c m", p=TILE))
            wa2_sb = singles.tile([DENSE, HEADS], bf16)
            nc.sync.dma_start(out=wa2_sb, in_=wa2.ap())
            beff_sb = singles.tile([DENSE, 1], f32)
            nc.sync.dma_start(out=beff_sb, in_=beff.ap())
            sel_sb = singles.tile([TILE, ntiles, G], bf16)
            nc.sync.dma_start(out=sel_sb, in_=selt.ap())
            # e values for the whole core stay resident in SBUF (2KB/part)
            e_all = singles.tile([TILE, ntiles, HEADS], f32)

            t0 = 0
            for g, tpg in enumerate(plan):
                n0 = t0 * TILE
                GN = tpg * TILE
                # --- group loads (big, efficient DMAs) ---
                ts_g = loads.tile([TILE, CS, GN], fp8, tag="ts",
                                  name=f"ts_{g}")
                nc.scalar.dma_start(out=ts_g,
                                    in_=xt_s_v[:, :, n0:n0 + GN])
                tq_g = loads.tile([TILE, CQ, GN], fp8, tag="tq",
                                  name=f"tq_{g}")
                nc.sync.dma_start(out=tq_g,
                                  in_=xt_q_v[:, :, n0:n0 + GN])
                ns_g = loads.tile([TILE, tpg, H], bf16, tag="ns",
                                  name=f"ns_{g}")
                nc.scalar.dma_start(out=ns_g, in_=xn_s_v[:, t0:t0 + tpg, :])
                nq_g = loads.tile([TILE, tpg, SEQ], bf16, tag="nq",
                                  name=f"nq_{g}")
                nc.sync.dma_start(out=nq_g, in_=xn_q_v[:, t0:t0 + tpg, :])

                # --- weighted-sum PSUM accumulators for this group ---
                pw = [ps_w.tile([HEADS * G, 512], f32, tag=f"pw{i}",
                                name=f"pw{i}_{g}")
                      for i in range(3)]

                hts = []
                for u in range(GN // 512):  # sub-blocks of 512 nodes
                    # scoresT[16, 512] = sum_c Wsc_c.T @ XT_c  (+ beff)
                    psc = ps_sc.tile([DENSE, 512], f32, tag="psc")
                    for c in range(NSC):
                        if c < CS:
                            rhs = ts_g[:, c, u * 512:(u + 1) * 512]
                        else:
                            rhs = tq_g[:, c - CS, u * 512:(u + 1) * 512]
                        nc.tensor.matmul(
                            psc, wsc_sb[:, c, :], rhs,
                            start=(c == 0), stop=(c == NSC - 1))
                    ht = small.tile([DENSE, 512], bf16, tag="ht")
                    nc.scalar.activation(out=ht, in_=psc,
                                         func=mybir.ActivationFunctionType.Tanh,
                                         bias=beff_sb)
                    hts.append(ht)

                for tl in range(tpg):  # per 128-node tile
                    t = t0 + tl
                    ht = hts[tl // 4]
                    j = tl % 4
                    # s[128, 4] = h @ W_a2 + b_a2
                    psn = ps_sn.tile([TILE, HEADS], f32, tag="psn")
                    nc.tensor.matmul(psn, ht[:, j * TILE:(j + 1) * TILE],
                                     wa2_sb, start=True, stop=True)
                    # e = exp(s) -> resident f32 buffer (also an output)
                    nc.scalar.activation(out=e_all[:, t, :], in_=psn,
                                         func=mybir.ActivationFunctionType.Exp)
                    # E2[n, (seg, h)] = e[n, h] * sel[n, seg]
                    e2 = small.tile([TILE, G, HEADS], bf16, tag="e2")
                    nc.vector.tensor_mul(
                        e2,
                        e_all[:, t, :].unsqueeze(1).broadcast_to(
                            [TILE, G, HEADS]),
                        sel_sb[:, t, :].unsqueeze(2).broadcast_to(
                            [TILE, G, HEADS]),
                    )
                    e2f = e2.rearrange("p a b -> p (a b)")
                    # Tpart[(seg, h), d] += E2.T @ Xnat
                    st = (tl == 0)
                    sp = (tl == tpg - 1)
                    nc.tensor.matmul(pw[0], e2f, ns_g[:, tl, :],
                                     start=st, stop=sp)
                    nc.tensor.matmul(pw[1], e2f, nq_g[:, tl, :512],
                                     start=st, stop=sp)
                    nc.tensor.matmul(pw[2], e2f, nq_g[:, tl, 512:],
                                     start=st, stop=sp)

                # --- evacuate group partial sums ---
                tv = evac.tile([HEADS * G, F], f32, tag="tv")
                for i in range(3):
                    nc.vector.tensor_copy(tv[:, i * 512:(i + 1) * 512], pw[i])
                nc.scalar.dma_start(out=tout.ap()[g], in_=tv)
                nc.scalar.dma_start(
                    out=e_out.ap()[:, t0:t0 + tpg, :],
                    in_=e_all[:, t0:t0 + tpg, :])
                t0 += tpg
                nc.scalar.dma_start(
                    out=e_out.ap()[:, t0:t0 + TPG, :],
                    in_=e_all[:, t0:t0 + TPG, :])


    nc.compile()
    return nc


# ----------------------------------------------------------------------------
# Device-output emulation (numpy) -- used for fallback and self-checking
# ----------------------------------------------------------------------------

def _emulate_core(in_map, NP, G):
    ntiles = NP // TILE
    plan = _group_plan(ntiles)
    ngroups = len(plan)
    xn = np.concatenate([in_map["xn_s"].astype(np.float32),
                         in_map["xn_q"].astype(np.float32)], -1)
    xt_s = in_map["xt_s"].astype(np.float32)
    xt_q = in_map["xt_q"].astype(np.float32)
    xt = np.concatenate([xt_s, xt_q], 0)            # [F, NP]
    wsc = in_map["wsc"].astype(np.float32)      # [128, 6, 2, 16]
    wsc = wsc.transpose(1, 2, 0, 3).reshape(F, DENSE) / 32.0
    wa2 = in_map["wa2"].astype(np.float32)
    beff = in_map["beff"].astype(np.float32)
    sel = in_map["sel"].astype(np.float32)          # [128, nt, G]
    sel_n = sel.transpose(1, 0, 2).reshape(NP, G)   # node-major

    scores = xt.T @ wsc + beff.reshape(1, DENSE)    # [NP, 16]
    h = np.tanh(scores).astype(BF16).astype(np.float32)
    s = h @ wa2
    e = np.exp(s).astype(np.float32)                # [NP, 4]
    e2 = (e[:, None, :] * sel_n[:, :, None]).astype(BF16).astype(np.float32)
    e2 = e2.reshape(NP, G * HEADS)
    tout = np.zeros((ngroups, HEADS * G, F), np.float32)
    lo = 0
    for g, tpg in enumerate(plan):
        gn = tpg * TILE
        tout[g] = e2[lo:lo + gn].T @ xn[lo:lo + gn]
        lo += gn
    e_out = e.reshape(ntiles, TILE, HEADS).transpose(1, 0, 2)
    return {"e_out": np.ascontiguousarray(e_out), "tout": tout}


# ----------------------------------------------------------------------------
# Main entry
# ----------------------------------------------------------------------------

def kernel(structure_feat, seq_feat, batch_id, num_graphs,
           W_proj, b_proj, W_a1, b_a1, W_a2, b_a2,
           W_f1, b_f1, W_f2, b_f2):
    global LAST_EXEC_TIME_NS, LAST_PROFILE

    struct = np.asarray(structure_feat, np.float32)
    seq = np.asarray(seq_feat, np.float32)
    bid = np.asarray(batch_id).astype(np.int64)
    B = int(num_graphs)
    W_proj = np.asarray(W_proj, np.float32)
    b_proj = np.asarray(b_proj, np.float32)
    W_a1 = np.asarray(W_a1, np.float32)
    b_a1 = np.asarray(b_a1, np.float32)
    W_a2 = np.asarray(W_a2, np.float32)
    b_a2 = np.asarray(b_a2, np.float32)

    N = struct.shape[0]
    # pad so each core gets an equal node count divisible into 1024-groups
    NP = -(-N // (NCORES * GROUP_NODES)) * GROUP_NODES
    npad = NCORES * NP - N
    if npad:
        struct = np.concatenate(
            [struct, np.zeros((npad, H), np.float32)], 0)
        seq = np.concatenate([seq, np.zeros((npad, SEQ), np.float32)], 0)
        bid_p = np.concatenate([bid, np.full(npad, -1, np.int64)])
    else:
        bid_p = bid

    NSC_ = F // TILE
    G, sel, seg2graph = _plan_segments(bid_p, NCORES, NP, B)
    ntiles = NP // TILE
    ngroups = len(_group_plan(ntiles))

    # fold W_proj / biases into the score weights (float64 for safety)
    W_comb = (W_proj.astype(np.float64) @ W_a1[H:].astype(np.float64))
    wsc_np = np.concatenate([W_a1[:H].astype(np.float64), W_comb], 0)
    beff_np = (b_a1.astype(np.float64)
               + b_proj.astype(np.float64) @ W_a1[H:].astype(np.float64))

    struct_bf = struct.astype(BF16)
    seq_bf = seq.astype(BF16)

    in_maps = []
    for c in range(NCORES):
        s0 = c * NP
        sl = slice(s0, s0 + NP)
        in_maps.append({
            "xn_s": struct_bf[sl],
            "xn_q": seq_bf[sl],
            "xt_s": np.ascontiguousarray(struct_bf[sl].T).astype(FP8),
            "xt_q": np.ascontiguousarray(seq_bf[sl].T).astype(FP8),
            "sel": np.ascontiguousarray(
                sel[c].reshape(ntiles, TILE, G).transpose(1, 0, 2)
            ).astype(BF16),
            "wsc": wsc_np.astype(BF16),
            "wa2": W_a2.astype(BF16),
            "beff": beff_np.reshape(DENSE, 1).astype(np.float32),
        })

    use_device = (
        os.environ.get("KERNEL_EMULATE", "0") != "1"
        and HEADS * G <= 128
    )
    results = None
    if use_device:
        from concourse.bass_utils import run_bass_kernel_spmd
        key = (NP, G)
        for attempt in range(3):
            try:
                if key not in _COMPILED:
                    _COMPILED[key] = _build(NP, G)
                nc = _COMPILED[key]
                res = run_bass_kernel_spmd(
                    nc, in_maps, core_ids=list(range(NCORES)),
                    trace=os.environ.get("KERNEL_TRACE", "0") == "1",
                )
                LAST_EXEC_TIME_NS = res.exec_time_ns
                LAST_PROFILE = res.profile_json
                results = res.results
                break
            except Exception as exc:  # device flake -> retry, then emulate
                sys.stderr.write(f"kernel: device attempt {attempt} "
                                 f"failed: {exc!r}\n")
                if attempt >= 1:
                    # second failure: rebuild from scratch before last try
                    _COMPILED.pop(key, None)
    if results is None:
        results = [_emulate_core(m, NP, G) for m in in_maps]

    # ---------------- host-side combine ----------------
    e_full = np.stack([r["e_out"] for r in results])        # [8,128,nt,4]
    e_full = e_full.transpose(0, 2, 1, 3).reshape(NCORES * NP, HEADS)[:N]
    denom = np.stack(
        [np.bincount(bid, weights=e_full[:, h].astype(np.float64),
                     minlength=B) for h in range(HEADS)], -1)  # [B,4]
    denom = np.maximum(denom, 1e-30)
    attn = (e_full / denom[bid]).astype(np.float32)

    T = np.zeros((B, HEADS, F), np.float64)
    for c in range(NCORES):
        tc_ = results[c]["tout"].astype(np.float64)  # [ngroups, 4G, F]
        for g in range(ngroups):
            for s_ in range(G):
                b = seg2graph[c, g, s_]
                if b >= 0:
                    T[b] += tc_[g, HEADS * s_:HEADS * (s_ + 1), :]

    inv = 1.0 / denom                                       # [B,4]
    pooled_s = np.einsum("bhd,bh->bd", T[:, :, :H], inv)
    u = np.einsum("bhd,bh->bd", T[:, :, H:], inv)
    # sum of attention weights per graph is HEADS for non-empty graphs, 0 for
    # empty ones (reference segment_sum yields exactly 0 there)
    counts = np.bincount(bid, minlength=B)
    wsum = (HEADS * (counts > 0).astype(np.float64))[:, None]
    pooled_q = u @ W_proj.astype(np.float64) + wsum * b_proj.astype(np.float64)
    pooled = np.concatenate([pooled_s, pooled_q], -1)       # [B, 2H]
    emb = _elu(pooled @ np.asarray(W_f1, np.float64) + np.asarray(b_f1))
    out = emb @ np.asarray(W_f2, np.float64) + np.asarray(b_f2)
    return out.astype(np.float32), attn
